# revision 40
# baseline (speedup 1.0000x reference)
"""Multi-head attention (B=4, S=2048, D=1024, H=16) on 8 TRN2 NeuronCores. v3

Sharding: data-parallel over batch (4) x tensor-parallel over heads (2 groups
of 8). Core c handles batch c//2, head-group c%2. Each core computes its
partial output projection (over its 512 head-dims); the two partials per
batch are summed on the host at gather time (the TP all-reduce).

Design (v2 base):
  - exp is SPLIT between ScalarE (ACT Exp) and VectorE (a Schraudolph
    bit-trick exp in one tensor_scalar op: bf16_bits = int16(ps*128/ln2 +
    (127-C)*128) through an int16 bitcast view; rel err ~ +-4%, mostly
    cancelling between softmax num and den).
  - the DVE carries the denominator accumulation (two 8-deep bf16 add
    chains); final dens come from M=1 col-tiled matmul pairs; the whole
    normalize tail is DEFERRED into the next window's j-loop at fixed j
    slots; per-window recip broadcast is 2 bf16 K=1 matmuls.
  - scores are K=64|64 row-tiled pairs, attnV M=64|64 col-tiled pairs;
    tile-disjoint pairs genuinely stream concurrently (~220ns per pair,
    same as ONE full matmul).

v3 additions (413us -> 375us):
  - PE work is emitted in MACRO-STEPS of two js: scores_j/scores_j+1
    adjacent, then both attnV pairs, then a 4-step projection-drip block.
    A shape switch costs ~110-130ns of PE dead time (no LDWEIGHTS
    pull-ahead across shapes on this toolchain), so same-shape neighbors
    matter: drip windows went 20.1 -> 17.85us.
  - preamble merge: the pair-0 kT/qT projection chains ride inside the
    V projection's first two PSUM-group k-loops (group 0 is DMA-paced, so
    the extra matmuls are free); wv|wk|wq are packed into ONE dram param
    so each d-chunk is a single DMA (issue serialization on the sync
    queue was pacing the preamble at ~640ns/DMA).
  - the output is written bf16 (host gather upcasts) and drained as one
    [128,1024] DMA per q-chunk; the first 6 q-chunks of the Wo projection
    are dripped into the last windows' free PE slots (their PSUM drains
    split across ScalarE/DVE so neither pacing engine absorbs both).

Known-dead ends (measured): den partials via per-j M=1 PE pairs (+58us:
in-situ pair slots cost ~310ns, and the PE paces the drip windows);
gpsimd accum-DMA den chains (queue drifts ~1 window behind, the fold and
the et-ring WAR then stall fast windows); gpsimd tensor_tensor (shares
the DVE SBUF port -- concurrent use poisons DVE throughput 3.6x);
--enable-ldw-opt=true (walrus codegen crashes on tiled LDWEIGHTS).
"""

import sys
import types

import numpy as np
import ml_dtypes

BF16 = ml_dtypes.bfloat16

D = 1024        # d_model
S = 2048        # sequence length
B = 4           # batch
NH = 16         # total heads
DK = 64         # head dim
HPC = 8         # heads per core
G = 512         # features per core (HPC * DK)
NCORES = 8
SCALE = 1.0 / np.sqrt(DK)

KC = D // 128   # 8 contraction chunks of 128
FC = G // 128   # 4 feature chunks per core (= head pairs)
SC = S // 128   # 16 seq chunks of 128
QW = 512        # q-window per head in the attention inner loop
NQW = S // QW   # 4
NJ = S // 128   # 16 key chunks
AD = 4          # attnV emission delay in j-steps (decouple PE from exp latency)

# js whose exp runs on the DVE via the bit-trick (rest on ScalarE); the
# DVE also carries the denominator accumulation, so it takes few exps
DVE_JS = frozenset((2, 7, 12))
# Schraudolph constants: exp(x) ~ bf16_frombits(int16(x*TSA + TSB))
_LN2 = float(np.log(2.0))
_SCHC = 0.04367744
TSA = float(128.0 / _LN2)
TSB = float((127.0 - _SCHC) * 128.0)


def _install_axon_profile_hook():
    """The image's antenv lacks axon_hooks; shim it so trace=True works."""
    import antenv

    if "antenv.axon_hooks" in sys.modules:
        return
    mod = types.ModuleType("antenv.axon_hooks")
    mod._hook = None

    def set_axon_ntff_profile_hook(h):
        mod._hook = h

    def get_axon_ntff_profile_hook():
        return mod._hook

    mod.set_axon_ntff_profile_hook = set_axon_ntff_profile_hook
    mod.get_axon_ntff_profile_hook = get_axon_ntff_profile_hook
    sys.modules["antenv.axon_hooks"] = mod
    antenv.axon_hooks = mod
    try:
        from trn_agent_boot.trn_boot import _ntff_profile_via_ctypes

        set_axon_ntff_profile_hook(
            _ntff_profile_via_ctypes("/opt/axon/libaxon_pjrt.so")
        )
    except Exception:
        pass


def _split_sync_waits(nc, maxw=1):
    """This walrus build rejects instructions carrying more than ~1 sync wait
    command. Hoist excess waits onto same-engine nop instructions placed
    immediately before the owner (the sequencer blocks on them in order, so
    semantics are preserved). Sem updates stay on the real instruction."""
    import concourse.mybir as mybir

    cnt = 0
    for f in nc.m.functions:
        for bb in f.blocks:
            new = []
            for inst in bb.instructions:
                si = getattr(inst, "sync_info", None)
                waits = list(si.on_wait) if si is not None else []
                if len(waits) > maxw:
                    extra, keep = waits[:-maxw], waits[-maxw:]
                    for i in range(0, len(extra), maxw):
                        nop = mybir.InstNoOp(name=f"wsplit-{cnt}", ins=[], outs=[])
                        cnt += 1
                        nop.engine = inst.engine
                        nop.sync_info = mybir.SyncInfo(
                            on_wait=extra[i : i + maxw], on_update=[]
                        )
                        new.append(nop)
                    inst.sync_info = mybir.SyncInfo(
                        on_wait=keep, on_update=list(si.on_update)
                    )
                new.append(inst)
            bb.instructions[:] = new


def build_nc():
    import concourse.bass as bass
    import concourse.mybir as mybir
    from concourse import tile

    f32 = mybir.dt.float32
    bf16 = mybir.dt.bfloat16
    i16 = mybir.dt.int16
    Exp = mybir.ActivationFunctionType.Exp
    Ln = mybir.ActivationFunctionType.Ln
    Mult = mybir.AluOpType.mult
    Add = mybir.AluOpType.add

    nc = bass.Bass()

    xT_d = nc.declare_dram_parameter("xT", [D, S], bf16, isOutput=False)
    # wv|wk|wq packed per d-chunk so each chunk is ONE dma (issue time on
    # the sync queue was pacing the preamble at ~640ns per dma)
    wkvq_d = nc.declare_dram_parameter("wkvq", [D, 3 * G], bf16, isOutput=False)
    woT_d = nc.declare_dram_parameter("woT", [128, FC * D], bf16, isOutput=False)
    bqk_d = nc.declare_dram_parameter("bqk", [128, 2 * FC], f32, isOutput=False)
    bv_d = nc.declare_dram_parameter("bv", [1, G], bf16, isOutput=False)
    # bf16 output halves the out-dma traffic; the host gather upcasts
    out_d = nc.declare_dram_parameter("out", [S, D], bf16, isOutput=True)

    with tile.TileContext(nc) as tc:
        with (
            tc.tile_pool(name="const", bufs=1) as cpool,
            tc.tile_pool(name="xt", bufs=1) as xpool,
            tc.tile_pool(name="wts", bufs=1) as wpool,
            tc.tile_pool(name="acts", bufs=1) as apool,
        ):
            # ---- constants / biases ----
            ones_bf = cpool.tile([1, 128], bf16, name="ones_bf")
            nc.vector.memset(ones_bf[:], 1.0)
            # ones column for den matmuls (lhsT [128,1])
            onec_bf = cpool.tile([128, 1], bf16, name="onec_bf")
            nc.vector.memset(onec_bf[:], 1.0)
            # zero lhsT row for bank-clearing matmuls
            zrow = cpool.tile([1, 128], bf16, name="zrow")
            nc.vector.memset(zrow[:], 0.0)
            # bf16 ones rows at partitions 0 and 32 for the recip broadcasts
            onesb = cpool.tile([33, 64], bf16, name="onesb")
            nc.vector.memset(onesb[0:1, :], 1.0)
            nc.vector.memset(onesb[32:33, :], 1.0)
            bqk_sb = cpool.tile([128, 2 * FC], f32, name="bqk_sb")
            bv_sb = cpool.tile([1, G], bf16, name="bv_sb")

            # ---- weight / input loads (interleaved so phase 1's k-loop
            # can start as soon as the first xT/wkvq chunks land) ----
            xT_sb, wqT_sb, wkT_sb, wvT_sb = [], [], [], []
            for k in range(KC):
                t = xpool.tile([128, S], bf16, name=f"xT{k}", tag=f"xT{k}")
                nc.sync.dma_start(out=t[:], in_=xT_d[128 * k : 128 * (k + 1), :])
                xT_sb.append(t)
                t = wpool.tile([128, 3 * G], bf16, name=f"wkvq{k}", tag=f"wkvq{k}")
                nc.sync.dma_start(out=t[:], in_=wkvq_d[128 * k : 128 * (k + 1), :])
                wvT_sb.append(t[:, 0:G])
                wkT_sb.append(t[:, G : 2 * G])
                wqT_sb.append(t[:, 2 * G : 3 * G])
                if k == 0:
                    nc.sync.dma_start(out=bqk_sb[:], in_=bqk_d[:])
                    nc.sync.dma_start(out=bv_sb[:], in_=bv_d[:])
            woTp = wpool.tile([128, FC * D], bf16, name="woTp", tag="woTp")
            nc.sync.dma_start(out=woTp[:], in_=woT_d[:])
            woT_sb = [woTp[:, m * D : (m + 1) * D] for m in range(FC)]

            # ---- persistent activations ----
            v_sb = [
                apool.tile([128, G], bf16, name=f"v{s}", tag=f"v{s}")
                for s in range(SC)
            ]
            qT_sb = [
                apool.tile([128, S], bf16, name=f"qT{m}", tag=f"qT{m}")
                for m in range(FC)
            ]
            kT_sb = [
                apool.tile([128, S], bf16, name=f"kT{m}", tag=f"kT{m}")
                for m in range(FC)
            ]
            # attention output per head PAIR [128, S]: head 2t rows 0-63,
            # head 2t+1 rows 64-127
            ao_sb = [
                apool.tile([128, S], bf16, name=f"ao{t}", tag=f"ao{t}")
                for t in range(FC)
            ]
            # exp-weight ring: PERSISTENT tiles, not a pool — pool-slot
            # reuse was serializing every exp behind the PREVIOUS et tile's
            # attnV/den readers (one j earlier) instead of the ring-distance
            # readers, stretching the j-loop period by ~15%
            NET = 5 + AD
            et_ring = [
                apool.tile([128, 2 * QW], bf16, name=f"etr{r}", tag=f"etr{r}")
                for r in range(NET)
            ]

            # ======== phase 1: V = x @ WvT + bv  (layout [seq, feat]) ========
            # k-outer over groups of 4 seq chunks so compute starts after the
            # first xT/wvT chunks land (JIT with the DMAs).
            with tc.tile_pool(name="pqkv", bufs=4, space="PSUM") as pq:
                # phases A/B: V sg-groups 0,1 carry the pair-0 kT / qT
                # projection chains in the other 4 psum banks -- group 0's
                # k-loop is DMA-paced (chunks still landing), so the kq0
                # matmuls ride in its PE slack instead of serializing after
                # phase 1 (phase 2 used to cost ~14us of idle ScalarE/DVE)
                for g, (nm, w_sb, dst_sb, bcol) in enumerate(
                    (("k", wkT_sb, kT_sb, FC), ("q", wqT_sb, qT_sb, 0))
                ):
                    sg = 4 * g
                    pvs = [
                        pq.tile([128, G], f32, name=f"pv{sg+i}", tag="pv")
                        for i in range(4)
                    ]
                    pps = [
                        pq.tile([128, 512], f32, name=f"p{nm}0_{qc}", tag="pk")
                        for qc in range(4)
                    ]
                    for k in range(KC):
                        for i in range(4):
                            s = sg + i
                            nc.tensor.matmul(
                                pvs[i][:],
                                lhsT=xT_sb[k][:, 128 * s : 128 * (s + 1)],
                                rhs=wvT_sb[k][:],
                                start=(k == 0),
                                stop=False,
                            )
                        for qc in range(4):
                            nc.tensor.matmul(
                                pps[qc][:],
                                lhsT=w_sb[k][:, 0:128],
                                rhs=xT_sb[k][:, 512 * qc : 512 * (qc + 1)],
                                start=(k == 0),
                                stop=(k == KC - 1),
                            )
                    for i in range(4):
                        nc.tensor.matmul(
                            pvs[i][:],
                            lhsT=ones_bf[:],
                            rhs=bv_sb[:],
                            start=False,
                            stop=True,
                        )
                        nc.vector.tensor_copy(v_sb[sg + i][:], pvs[i][:])
                    for qc in range(4):
                        nc.vector.tensor_scalar_add(
                            dst_sb[0][:, 512 * qc : 512 * (qc + 1)],
                            pps[qc][:],
                            bqk_sb[:, bcol : bcol + 1],
                        )

                # phase C: V sg-groups 2,3
                for sg in range(8, SC, 4):
                    pvs = [
                        pq.tile([128, G], f32, name=f"pv{sg+i}", tag="pv")
                        for i in range(4)
                    ]
                    for k in range(KC):
                        for i in range(4):
                            s = sg + i
                            nc.tensor.matmul(
                                pvs[i][:],
                                lhsT=xT_sb[k][:, 128 * s : 128 * (s + 1)],
                                rhs=wvT_sb[k][:],
                                start=(k == 0),
                                stop=False,
                            )
                    for i in range(4):
                        nc.tensor.matmul(
                            pvs[i][:],
                            lhsT=ones_bf[:],
                            rhs=bv_sb[:],
                            start=False,
                            stop=True,
                        )
                        nc.vector.tensor_copy(v_sb[sg + i][:], pvs[i][:])

            # ======== phase 3: attention, head pairs ========
            with (
                tc.tile_pool(name="ps", bufs=2, space="PSUM") as psp,
                tc.tile_pool(name="po", bufs=2, space="PSUM") as pop,
                tc.tile_pool(name="pd", bufs=1, space="PSUM") as pdp,
                tc.tile_pool(name="pbp", bufs=1, space="PSUM") as pbp,
                tc.tile_pool(name="dn", bufs=2) as dnp,
            ):
                pending = []  # deferred normalize tails

                def emit_tail(state, step):
                    """One slice of window state's deferred normalize tail.
                    Steps are spread across the NEXT window's j-loop so no
                    engine sees a burst at the window boundary."""
                    pt, pw, po, accs, hold = state
                    pqs = slice(QW * pw, QW * (pw + 1))
                    if step == 0:
                        # final denominators from the partial exp sums:
                        # col-tiled concurrent M=1 pairs, den A at partition
                        # 0 and den B at partition 32, one pair per partial
                        # accumulator (2 DMA chains + the DVE's accB)
                        pd = pdp.tile([33, QW], f32, name=f"pd{pt}_{pw}", tag="pd")
                        for ai, acc in enumerate(accs):
                            first = ai == 0
                            last = ai == len(accs) - 1
                            nc.tensor.matmul(
                                pd[0:1, :],
                                lhsT=onec_bf[:],
                                rhs=acc[:, 0:QW],
                                start=first,
                                stop=last,
                                tile_position=(0, 0),
                                skip_group_check=True,
                            )
                            nc.tensor.matmul(
                                pd[32:33, :],
                                lhsT=onec_bf[:],
                                rhs=acc[:, QW : 2 * QW],
                                start=first,
                                stop=last,
                                tile_position=(0, 32),
                                skip_group_check=True,
                            )
                        hold["pd"] = pd
                    elif step == 1:
                        # ln(den): one slab op covers den A (partition 0) and
                        # den B (partition 32); partitions 1-31 are garbage
                        # that nothing downstream reads
                        drl = dnp.tile([33, QW], f32, name=f"dl{pt}_{pw}", tag="dl")
                        nc.scalar.activation(drl[:], hold["pd"][:], Ln)
                        hold["drl"] = drl
                    elif step == 2:
                        # 1/den = exp(-ln den), emitted as bf16 so the
                        # broadcast matmuls get a fast bf16 rhs
                        dr = dnp.tile([33, QW], bf16, name=f"dr{pt}_{pw}", tag="dr")
                        nc.scalar.activation(dr[:], hold["drl"][:], Exp, scale=-1.0)
                        hold["dr"] = dr
                    elif step == 3:
                        # broadcast recips to partitions 0-63 (A) / 64-127 (B).
                        # pb has its OWN psum bank: sharing the ps pool shifts
                        # the 2-buffer rotation parity and couples scores_j to
                        # exp_{j-1} instead of exp_{j-2}
                        pb = pbp.tile([128, QW], f32, name=f"pb{pt}_{pw}", tag="pb")
                        nc.tensor.matmul(
                            pb[0:64, :],
                            lhsT=onesb[0:1, :],
                            rhs=hold["dr"][0:1, :],
                            start=True,
                            stop=True,
                            tile_position=(0, 0),
                            skip_group_check=True,
                        )
                        nc.tensor.matmul(
                            pb[64:128, :],
                            lhsT=onesb[32:33, :],
                            rhs=hold["dr"][32:33, :],
                            start=True,
                            stop=True,
                            tile_position=(32, 64),
                            skip_group_check=True,
                        )
                        hold["pb"] = pb
                    elif step == 4:
                        pbs = dnp.tile([128, QW], f32, name=f"pbs{pt}_{pw}", tag="pbs")
                        nc.vector.tensor_copy(pbs[:], hold["pb"][:])
                        hold["pbs"] = pbs
                    elif step == 5:
                        nc.vector.tensor_mul(ao_sb[pt][:, pqs], po[:], hold["pbs"][:])

                # tail step -> j slot in the next window's loop
                TAIL_AT = {0: 0, 1: 1, 2: 2, 3: 3, 4: 4, 5: 5}
                NSTEP = 6

                # deferred qk projections for head pairs 1-3, drip-fed into
                # the j-loops: each chain cycles the pb bank (free outside
                # tail js 3-4), 8 accumulating matmuls + one bias-add drain
                def qk_chain_steps():
                    for m in range(1, FC):
                        for w_sb, dst_sb, bcol in (
                            (wkT_sb, kT_sb, FC),
                            (wqT_sb, qT_sb, 0),
                        ):
                            hold = {}
                            for qc in range(4):
                                for k in range(KC):
                                    yield (w_sb, dst_sb, bcol, m, qc, k, hold)
                                yield (w_sb, dst_sb, bcol, m, qc, None, hold)

                def emit_qk_step(step):
                    w_sb, dst_sb, bcol, m, qc, k, hold = step
                    if k is None:
                        nc.vector.tensor_copy(
                            dst_sb[m][:, 512 * qc : 512 * (qc + 1)],
                            hold["ps"][:],
                        )
                        return
                    if k == 0:
                        hold["ps"] = pbp.tile(
                            [128, 512], f32, name=f"pqk{m}{qc}_{bcol}", tag="pb"
                        )
                    nc.tensor.matmul(
                        hold["ps"][:],
                        lhsT=w_sb[k][:, 128 * m : 128 * (m + 1)],
                        rhs=xT_sb[k][:, 512 * qc : 512 * (qc + 1)],
                        start=(k == 0),
                        stop=(k == KC - 1),
                    )

                qk_gen = qk_chain_steps()

                # Wo output-projection units dripped into the late windows'
                # free drip slots (the qk chains exhaust at window 12, and
                # ao[3] for q-window w is finalized during window 13+w).
                # One unit = one q chunk: 8 matmuls + 2 drains + 1 out-DMA.
                wo_state = {"qc": 0, "step": 0, "oc": None, "ps": None}
                wo_done = set()

                def emit_wo_step(gw):
                    qc = wo_state["qc"]
                    if qc >= SC:
                        return False
                    s = wo_state["step"]
                    if s == 0 and qc // 4 > gw - 13:
                        return False  # ao for this q-window not final yet
                    e, ke = divmod(s, 5)
                    if ke == 0:
                        wo_state["ps"] = pbp.tile(
                            [128, 512], f32, name=f"pwd{qc}_{e}", tag="pb"
                        )
                        if e == 0:
                            wo_state["oc"] = dnp.tile(
                                [128, 1024], bf16, name=f"ocd{qc}", tag="ocd"
                            )
                    if ke < 4:
                        nc.tensor.matmul(
                            wo_state["ps"][:],
                            lhsT=ao_sb[ke][:, 128 * qc : 128 * (qc + 1)],
                            rhs=woT_sb[ke][:, 512 * e : 512 * (e + 1)],
                            start=(ke == 0),
                            stop=(ke == FC - 1),
                        )
                    else:
                        # drain copies split across ScalarE/DVE: the late
                        # windows are engine-paced, so one engine must not
                        # absorb both
                        if e == 0:
                            nc.scalar.copy(
                                wo_state["oc"][:, 0:512], wo_state["ps"][:]
                            )
                        else:
                            nc.vector.tensor_copy(
                                wo_state["oc"][:, 512:1024], wo_state["ps"][:]
                            )
                        if e == 1:
                            nc.sync.dma_start(
                                out=out_d[128 * qc : 128 * (qc + 1), :],
                                in_=wo_state["oc"][:],
                            )
                            wo_done.add(qc)
                            wo_state["qc"] = qc + 1
                            wo_state["step"] = 0
                            return True
                    wo_state["step"] = s + 1
                    return True

                gj = 0  # global j counter -> et ring slot
                for t in range(FC):
                    for w in range(NQW):
                        qs = slice(QW * w, QW * (w + 1))
                        # po: head A rows 0-63, head B rows 64-127
                        po = pop.tile([128, QW], f32, name=f"po{t}_{w}", tag="po")
                        # bf16 running sums of the exp tiles (two 8-deep
                        # chains bound the bf16 accumulation error); the
                        # denominators come from one matmul pair per window
                        # instead of a per-j M=1 matmul pair on the PE
                        accA = dnp.tile([128, 2 * QW], bf16, name=f"aA{t}_{w}", tag="aA")
                        accB = dnp.tile([128, 2 * QW], bf16, name=f"aB{t}_{w}", tag="aB")
                        ets = {}
                        # macro-steps of TWO js: same-shape PE work is
                        # emitted adjacently (scores_j, scores_j+1, then both
                        # attnV pairs, then the drip block) so the second op
                        # of each block streams switch-free -- each shape
                        # switch costs ~110-130ns of PE dead time (no LDW
                        # pull-ahead across shapes on this toolchain)
                        for jj in range(0, NJ + AD, 2):
                            # scores block
                            for j in (jj, jj + 1):
                                if j >= NJ:
                                    continue
                                ps = psp.tile(
                                    [128, 2 * QW], f32, name=f"ps{t}{w}{j}", tag="ps"
                                )
                                nc.tensor.matmul(
                                    ps[:, 0:QW],
                                    lhsT=kT_sb[t][0:64, 128 * j : 128 * (j + 1)],
                                    rhs=qT_sb[t][0:64, qs],
                                    start=True,
                                    stop=True,
                                    tile_position=(0, 0),
                                )
                                nc.tensor.matmul(
                                    ps[:, QW : 2 * QW],
                                    lhsT=kT_sb[t][64:128, 128 * j : 128 * (j + 1)],
                                    rhs=qT_sb[t][64:128, qs],
                                    start=True,
                                    stop=True,
                                    tile_position=(64, 0),
                                )
                                et = et_ring[gj % NET]
                                gj += 1
                                if j in DVE_JS:
                                    # Schraudolph bit-exp on the DVE: bf16
                                    # bits computed as int16(ps*TSA + TSB)
                                    nc.vector.tensor_scalar(
                                        et[:].bitcast(i16),
                                        ps[:],
                                        TSA,
                                        TSB,
                                        Mult,
                                        Add,
                                    )
                                else:
                                    nc.scalar.activation(et[:], ps[:], Exp)
                                ets[j] = et
                            # attnV block, delayed AD steps: the in-order PE
                            # consumes exps finished a full period ago
                            for j in (jj, jj + 1):
                                ja = j - AD
                                if ja < 0:
                                    continue
                                eta = ets[ja]
                                first, last = ja == 0, ja == NJ - 1
                                nc.tensor.matmul(
                                    po[0:64, :],
                                    lhsT=v_sb[ja][:, 128 * t : 128 * t + 64],
                                    rhs=eta[:, 0:QW],
                                    start=first,
                                    stop=last,
                                    tile_position=(0, 0),
                                    skip_group_check=True,
                                )
                                nc.tensor.matmul(
                                    po[64:128, :],
                                    lhsT=v_sb[ja][:, 128 * t + 64 : 128 * (t + 1)],
                                    rhs=eta[:, QW : 2 * QW],
                                    start=first,
                                    stop=last,
                                    tile_position=(0, 64),
                                    skip_group_check=True,
                                )
                            for j in (jj, jj + 1):
                                # denominator accumulation, one j behind the
                                # exp: js < NDMA ride idle DMA hardware
                                # (gpsimd accum-DMA), the rest the DVE
                                jaa = j - 1
                                if 0 <= jaa < NJ:
                                    src = ets[jaa]
                                    if jaa == 0:
                                        nc.vector.tensor_copy(accA[:], src[:])
                                    elif jaa < NJ // 2:
                                        nc.vector.tensor_add(accA[:], accA[:], src[:])
                                    elif jaa == NJ // 2:
                                        nc.vector.tensor_copy(accB[:], src[:])
                                    else:
                                        nc.vector.tensor_add(
                                            accB[:], accB[:], src[:]
                                        )
                                if pending:
                                    for sstep, sj in TAIL_AT.items():
                                        if j == sj:
                                            emit_tail(pending[0], sstep)
                                    if j == TAIL_AT[NSTEP - 1]:
                                        pending.pop(0)
                            # drip the deferred qk projections into PE slack
                            # (after the tail's pb use so the pool order
                            # keeps pb ahead of the chains each window).
                            # js 6-14 x2 = 18 steps = exactly 2 whole chains,
                            # so no chain ever spans a window boundary (a
                            # spanning chain would deadlock the in-order PE
                            # against the next tail's pb pool WAR)
                            ndrip = sum(1 for j in (jj, jj + 1) if 6 <= j <= 14)
                            for _ in range(2 * ndrip):
                                step = next(qk_gen, None)
                                if step is not None:
                                    emit_qk_step(step)
                                elif not emit_wo_step(t * NQW + w):
                                    break
                        pending.append((t, w, po, (accA, accB), {}))
                # drain the last window's tail
                st = pending.pop(0)
                for sstep in range(NSTEP):
                    emit_tail(st, sstep)

            # ======== phase 4: out = attn_out @ WoT (partial over G) ========
            with (
                tc.tile_pool(name="pwo", bufs=4, space="PSUM") as pwo,
                tc.tile_pool(name="ost", bufs=4) as ost,
            ):
                for qc in range(SC):
                    if qc in wo_done:
                        continue
                    oc = ost.tile([128, 1024], bf16, name=f"oc{qc}", tag="oc")
                    for e in range(2):
                        ps = pwo.tile([128, 512], f32, name=f"pw{qc}_{e}", tag="pw")
                        for m in range(FC):
                            nc.tensor.matmul(
                                ps[:],
                                lhsT=ao_sb[m][:, 128 * qc : 128 * (qc + 1)],
                                rhs=woT_sb[m][:, 512 * e : 512 * (e + 1)],
                                start=(m == 0),
                                stop=(m == FC - 1),
                            )
                        nc.vector.tensor_copy(
                            oc[:, 512 * e : 512 * (e + 1)], ps[:]
                        )
                    nc.sync.dma_start(
                        out=out_d[128 * qc : 128 * (qc + 1), :], in_=oc[:]
                    )

    _split_sync_waits(nc)
    return nc


_NC = None


def _get_nc():
    global _NC
    if _NC is None:
        _NC = build_nc()
    return _NC


def make_in_maps(x, Wq, bq, Wk, bk, Wv, bv, Wo, bo):
    x = np.asarray(x, np.float32)
    xT = [np.ascontiguousarray(x[b].T).astype(BF16) for b in range(B)]
    per_g = []
    for g in range(2):
        gs = slice(G * g, G * (g + 1))
        wqT = np.ascontiguousarray((np.asarray(Wq, np.float32)[gs] * SCALE).T)
        wkT = np.ascontiguousarray(np.asarray(Wk, np.float32)[gs].T)
        wvT = np.ascontiguousarray(np.asarray(Wv, np.float32)[gs].T)
        wkvq = np.concatenate([wvT, wkT, wqT], axis=1).astype(BF16)
        woT = np.ascontiguousarray(np.asarray(Wo, np.float32)[:, gs].T)
        woTp = np.concatenate(
            [woT[128 * m : 128 * (m + 1), :] for m in range(FC)], axis=1
        ).astype(BF16)
        bqk = np.empty((128, 2 * FC), np.float32)
        bqk[:, :FC] = (np.asarray(bq, np.float32)[gs] * SCALE).reshape(FC, 128).T
        bqk[:, FC:] = np.asarray(bk, np.float32)[gs].reshape(FC, 128).T
        bvv = np.asarray(bv, np.float32)[gs].reshape(1, G).astype(BF16)
        per_g.append(dict(wkvq=wkvq, woT=woTp, bqk=bqk, bv=bvv))
    in_maps = []
    for c in range(NCORES):
        b, g = c // 2, c % 2
        m = dict(per_g[g])
        m["xT"] = xT[b]
        in_maps.append(m)
    return in_maps


def run_cores(in_maps, trace=False):
    from concourse.bass_utils import run_bass_kernel_spmd

    if trace:
        _install_axon_profile_hook()
    nc = _get_nc()
    return run_bass_kernel_spmd(nc, in_maps, list(range(NCORES)), trace=trace)


def kernel(x, Wq, bq, Wk, bk, Wv, bv, Wo, bo, _trace=False, _want_res=False):
    in_maps = make_in_maps(x, Wq, bq, Wk, bk, Wv, bv, Wo, bo)
    res = run_cores(in_maps, trace=_trace)
    bo = np.asarray(bo, np.float32)
    out = np.empty((B, S, D), np.float32)
    for b in range(B):
        out[b] = (
            res.results[2 * b]["out"].astype(np.float32)
            + res.results[2 * b + 1]["out"].astype(np.float32)
            + bo
        )
    if _want_res:
        return out, res
    return out

